# revision 1
# baseline (speedup 1.0000x reference)
"""Multi-head causal attention (B=4, S=2048, D=1024, H=16) on 8 trn2 cores.

Sharding: tensor-parallel over heads x data-parallel over batch.
core c -> (batch b = c//2, head-group hg = c%2 of 8 heads). Every core runs
an identical SPMD program on different data:
  - QKV projections for its 512 features (8 heads). K kept transposed
    [feat, seq] in SBUF, V kept [seq, feat] with an appended ones column per
    head (softmax denominators come free out of the PV matmul), Q produced
    per 512-query superblock just in time.
  - Causal attention per (head, superblock): S^T = K^T.T @ Q^T per 128-key
    block as two N=256 matmuls (f32r K=64/M=128 is half-rate at N=512), exp
    on ScalarE with no max subtraction (scores are O(5), exp cannot
    overflow), 0/1 mask multiply on diagonal blocks, PV accumulation in
    PSUM with an M=128-padded stationary.
  - Output projection against the head-group's 512-column slice of Wo.
Host sums the two partial outputs per batch (the "all-reduce after W_o"
done at gather time) and folds the Wo @ bv + bo constant.

Two trn2-specific tricks:
  - All matmuls run in float32r (11-bit mantissa, 4x fp32 PE rate); inputs
    are pre-rounded on the host (RNE at bit 12) so DMA feeds matmul tiles
    directly.
  - The PE HAM clock-gate does not count K=64 matmuls as "busy", so a pure
    attention phase runs at 1.2 GHz. The projection and output-projection
    chains (K=128) are therefore interleaved INTO the attention stream,
    which keeps the clock at 2.4 GHz: projections for superblock sc+1 and
    the output projection for sc-1 are emitted between attention batches
    of superblock sc.
"""

import sys

import numpy as np

_BASS_PATH = "/opt/trn_rl_repo"
if _BASS_PATH not in sys.path:
    sys.path.insert(0, _BASS_PATH)

B, S, D, H, DK = 4, 2048, 1024, 16, 64
NCORES = 8
FH = 512  # features per core (8 heads)
HL = 8  # local heads
NSC = 4  # seq superblocks of 512
SQ = 512
NKB = 16  # key blocks of 128
NDM = 8  # d_model chunks of 128

_cache = {}


def _round_f32r(x: np.ndarray) -> np.ndarray:
    """Round fp32 to fp32r (RNE to 11 mantissa bits) - matches TRN2 HW."""
    v = np.ascontiguousarray(x, dtype=np.float32).view(np.uint32)
    lsb = (v >> np.uint32(12)) & np.uint32(1)
    out = ((v + np.uint32(0x7FF) + lsb) >> np.uint32(12)) << np.uint32(12)
    return out.view(np.float32)


def _build():
    import concourse.bacc as bacc
    import concourse.mybir as mybir
    from concourse.tile import TileContext

    f32, f32r = mybir.dt.float32, mybir.dt.float32r
    AF = mybir.ActivationFunctionType

    nc = bacc.Bacc("TRN2", target_bir_lowering=False, debug=False, num_devices=1)

    xq_d = nc.dram_tensor("xq", [D, S], f32r, kind="ExternalInput").ap()
    xk_d = nc.dram_tensor("xk", [D, S], f32r, kind="ExternalInput").ap()
    xv_d = nc.dram_tensor("xv", [D, S], f32r, kind="ExternalInput").ap()
    wq_d = nc.dram_tensor("wq", [D, FH], f32r, kind="ExternalInput").ap()
    wk_d = nc.dram_tensor("wk", [D, FH], f32r, kind="ExternalInput").ap()
    wv_d = nc.dram_tensor("wv", [D, FH], f32r, kind="ExternalInput").ap()
    wo_d = nc.dram_tensor("wo", [FH, D], f32r, kind="ExternalInput").ap()
    # master causal mask [128, 896]: m[k, c] = 1 iff k <= c - 384.
    # mask_j (j = diag block index) = master[:, (3-j)*128 : (3-j)*128+512]
    mask_d = nc.dram_tensor("masks", [128, 896], f32r, kind="ExternalInput").ap()
    bq_d = nc.dram_tensor("bq", [FH], f32, kind="ExternalInput").ap()
    bk_d = nc.dram_tensor("bk", [FH], f32, kind="ExternalInput").ap()
    out_d = nc.dram_tensor("out", [S, D], f32, kind="ExternalOutput").ap()

    with TileContext(nc) as tc:
        with (
            tc.tile_pool(name="res", bufs=1) as res,
            tc.tile_pool(name="st", bufs=1) as st,
            tc.tile_pool(name="psum", bufs=1, space="PSUM") as psp,
            tc.tile_pool(name="dram", bufs=1, space="DRAM") as dpool,
        ):
            kt = [res.tile([128, S], f32r, name=f"kt{i}", tag=f"kt{i}") for i in range(4)]
            # 520 data cols (8 heads x (64 V + ones)) + pad so PV can read a
            # 128-wide stationary slice for head 7 (rows 65+ of the PV output
            # are garbage and ignored)
            vaug = [
                res.tile([128, 584], f32r, name=f"va{k}", tag=f"va{k}")
                for k in range(NKB)
            ]
            master = res.tile([128, 896], f32r, name="master", tag="master")
            nc.sync.dma_start(master[:], mask_d[:])
            bq_t = [res.tile([128, 1], f32, name=f"bq{i}", tag=f"bq{i}") for i in range(4)]
            bk_t = [res.tile([128, 1], f32, name=f"bk{i}", tag=f"bk{i}") for i in range(4)]
            for i in range(4):
                nc.sync.dma_start(
                    bq_t[i][:],
                    bq_d[i * 128 : (i + 1) * 128].rearrange("(p o) -> p o", o=1),
                )
                nc.sync.dma_start(
                    bk_t[i][:],
                    bk_d[i * 128 : (i + 1) * 128].rearrange("(p o) -> p o", o=1),
                )
            ones_t = res.tile([128, HL], f32, name="ones", tag="ones")
            nc.vector.memset(ones_t[:], 1.0)
            wo_sb = []
            for fc in range(4):
                wt = res.tile([128, D], f32r, name=f"wo{fc}", tag=f"wo{fc}")
                nc.sync.dma_start(wt[:], wo_d[fc * 128 : (fc + 1) * 128, :])
                wo_sb.append(wt)
            ctxd = dpool.tile([FH, S], f32r, name="ctxd", tag="ctxd")

            qsf_box = {}  # sc -> [4 q-slice tiles]

            def make_proj_thunks(sc):
                thunks = []
                for pname, x_d, w_d in (
                    ("k", xk_d, wk_d),
                    ("v", xv_d, wv_d),
                    ("q", xq_d, wq_d),
                ):
                    box = {}

                    def load(pname=pname, x_d=x_d, w_d=w_d, box=box):
                        w_sb, xr = [], []
                        for dm in range(NDM):
                            wt = st.tile(
                                [128, FH], f32r, name=f"w{dm}", tag=f"w{dm}", bufs=2
                            )
                            nc.sync.dma_start(wt[:], w_d[dm * 128 : (dm + 1) * 128, :])
                            w_sb.append(wt)
                            xt = st.tile(
                                [128, SQ], f32r, name=f"x{dm}", tag=f"x{dm}", bufs=1
                            )
                            nc.sync.dma_start(
                                xt[:],
                                x_d[dm * 128 : (dm + 1) * 128, sc * SQ : (sc + 1) * SQ],
                            )
                            xr.append(xt)
                        box["w"], box["x"] = w_sb, xr
                        if pname == "q":
                            qsf_box[sc] = [
                                st.tile(
                                    [128, SQ], f32r, name=f"qs{i}", tag=f"qs{i}", bufs=2
                                )
                                for i in range(4)
                            ]

                    for gi in range(4):

                        def group(pname=pname, gi=gi, box=box, sc=sc, load=load):
                            if gi == 0:
                                load()
                            w_sb, xr = box["w"], box["x"]
                            if pname in ("q", "k"):
                                pp = psp.tile(
                                    [128, SQ], f32, name="pp", tag="pp", bufs=2
                                )
                                for dm in range(NDM):
                                    nc.tensor.matmul(
                                        pp[:],
                                        w_sb[dm][:, gi * 128 : (gi + 1) * 128],
                                        xr[dm][:],
                                        start=(dm == 0),
                                        stop=(dm == NDM - 1),
                                    )
                                if pname == "k":
                                    nc.scalar.activation(
                                        kt[gi][:, sc * SQ : (sc + 1) * SQ],
                                        pp[:],
                                        AF.Identity,
                                        bias=bk_t[gi][:],
                                    )
                                else:
                                    nc.scalar.activation(
                                        qsf_box[sc][gi][:],
                                        pp[:],
                                        AF.Identity,
                                        bias=bq_t[gi][:],
                                    )
                            else:  # v
                                kb = sc * 4 + gi
                                pp = psp.tile(
                                    [128, FH], f32, name="pp", tag="pp", bufs=2
                                )
                                for dm in range(NDM):
                                    nc.tensor.matmul(
                                        pp[:],
                                        xr[dm][:, gi * 128 : (gi + 1) * 128],
                                        w_sb[dm][:],
                                        start=(dm == 0),
                                        stop=(dm == NDM - 1),
                                    )
                                va3 = vaug[kb][:, 0 : HL * 65].rearrange(
                                    "p (h e) -> p h e", e=65
                                )
                                pp3 = pp[:].rearrange("p (h e) -> p h e", e=64)
                                nc.vector.tensor_copy(va3[:, :, 0:64], pp3[:])
                                nc.vector.tensor_copy(
                                    va3[:, :, 64:65],
                                    ones_t[:].rearrange("p (h o) -> p h o", o=1),
                                )

                        thunks.append(group)
                return thunks

            def make_attn_batches(h, sb):
                """Return list of batch thunks for one (head, superblock)."""
                ti, po = h // 2, (h % 2) * 64
                nkb = 4 * (sb + 1)
                kbs = list(range(4 * sb, 4 * sb + 4)) + list(range(4 * sb))
                state = {}

                def batch(b0):
                    if b0 == 0:
                        state["cp"] = psp.tile(
                            [128, SQ], f32, name="cp", tag="cp", bufs=2
                        )
                        state["emitted"] = 0
                    cp = state["cp"]
                    group = []
                    for i in range(b0, b0 + 4):
                        kb = kbs[i]
                        sp = psp.tile([128, SQ], f32, name="sp", tag="sp", bufs=4)
                        for n0 in (0, 256):
                            nc.tensor.matmul(
                                sp[:, n0 : n0 + 256],
                                kt[ti][po : po + 64, kb * 128 : (kb + 1) * 128],
                                qsf_box[sb][ti][po : po + 64, n0 : n0 + 256],
                                start=True,
                                stop=True,
                            )
                        es = st.tile([128, SQ], f32r, name="es", tag="es", bufs=5)
                        nc.scalar.activation(es[:], sp[:], AF.Exp)
                        if kb >= sb * 4:
                            j = kb - sb * 4
                            es2 = st.tile(
                                [128, SQ], f32r, name="es2", tag="es2", bufs=5
                            )
                            nc.vector.tensor_mul(
                                es2[:],
                                es[:],
                                master[:, (3 - j) * 128 : (3 - j) * 128 + 512],
                            )
                            es = es2
                        group.append((kb, es))
                    for off in reversed(range(4)):
                        kb, es = group[off]
                        nc.tensor.matmul(
                            cp[:],
                            vaug[kb][:, h * 65 : h * 65 + 128],
                            es[:],
                            start=(state["emitted"] == 0),
                            stop=(state["emitted"] == nkb - 1),
                        )
                        state["emitted"] += 1
                    if b0 + 4 >= nkb:
                        # normalize and spill ctx^T slice to DRAM
                        d1 = st.tile([1, SQ], f32, name="d1", tag="d1", bufs=2)
                        nc.scalar.copy(d1[:], cp[64:65, :])
                        rb = st.tile([64, SQ], f32, name="rb", tag="rb", bufs=2)
                        nc.gpsimd.partition_broadcast(rb[:], d1[:])
                        rc = st.tile([64, SQ], f32, name="rc", tag="rc", bufs=2)
                        nc.vector.reciprocal_approx_fast(rc[:], rb[:])
                        nrm = st.tile([64, SQ], f32r, name="nrm", tag="nrm", bufs=2)
                        nc.vector.tensor_mul(nrm[:], cp[0:64, :], rc[:])
                        nc.sync.dma_start(
                            ctxd[h * 64 : (h + 1) * 64, sb * SQ : (sb + 1) * SQ],
                            nrm[:],
                        )

                return [
                    (lambda b0=b0: batch(b0)) for b0 in range(0, nkb, 4)
                ]

            def make_o_thunks(sb):
                thunks = []
                box = {}

                def load(sb=sb, box=box):
                    cfc = []
                    for fc in range(4):
                        ct = st.tile(
                            [128, SQ], f32r, name=f"cf{fc}", tag=f"cf{fc}", bufs=1
                        )
                        nc.sync.dma_start(
                            ct[:],
                            ctxd[fc * 128 : (fc + 1) * 128, sb * SQ : (sb + 1) * SQ],
                        )
                        cfc.append(ct)
                    box["c"] = cfc

                for qb in range(4):
                    for n2 in range(2):

                        def group(qb=qb, n2=n2, sb=sb, box=box):
                            if qb == 0 and n2 == 0:
                                load()
                            cfc = box["c"]
                            pp = psp.tile([128, SQ], f32, name="pp", tag="pp", bufs=2)
                            for fc in range(4):
                                nc.tensor.matmul(
                                    pp[:],
                                    cfc[fc][:, qb * 128 : (qb + 1) * 128],
                                    wo_sb[fc][:, n2 * SQ : (n2 + 1) * SQ],
                                    start=(fc == 0),
                                    stop=(fc == 3),
                                )
                            ob = st.tile([128, SQ], f32, name="ob", tag="ob", bufs=2)
                            nc.vector.tensor_copy(ob[:], pp[:])
                            nc.sync.dma_start(
                                out_d[
                                    sb * SQ + qb * 128 : sb * SQ + (qb + 1) * 128,
                                    n2 * SQ : (n2 + 1) * SQ,
                                ],
                                ob[:],
                            )

                        thunks.append(group)
                return thunks

            dummy_state = {"n": 0}

            def make_dummy_thunks(n):
                thunks = []
                for _ in range(n):

                    def g():
                        dp = psp.tile([128, SQ], f32, name="dp", tag="pp", bufs=2)
                        for t in range(4):
                            nc.tensor.matmul(
                                dp[:],
                                wo_sb[t][:, 0:128],
                                wo_sb[(t + 1) % 4][:, 0:SQ],
                                start=(t == 0),
                                stop=(t == 3),
                            )

                    thunks.append(g)
                return thunks

            # ---- emission schedule ----
            for t in make_proj_thunks(0):
                t()
            for sb in range(NSC):
                batches = []
                for h in range(HL):
                    batches += make_attn_batches(h, sb)
                warm = []
                if sb < NSC - 1:
                    warm += make_proj_thunks(sb + 1)
                if sb >= 1:
                    warm += make_o_thunks(sb - 1)
                # pad the warm stream so ~1 in 4 PE chains is K=128 (keeps
                # the HAM clock-gate at full rate through the attention tail)
                want = (len(batches) - len(warm)) // 3
                if want > 0:
                    warm += make_dummy_thunks(want)
                    # re-spread: alternate real and dummy warm items
                    real = warm[: len(warm) - want]
                    dum = warm[len(warm) - want :]
                    mixed = []
                    di = 0
                    for i, w in enumerate(real):
                        mixed.append(w)
                        while di < len(dum) and (di + 1) * len(real) <= (i + 1) * len(dum):
                            mixed.append(dum[di])
                            di += 1
                    mixed += dum[di:]
                    warm = mixed
                nb, nw = len(batches), len(warm)
                wi = 0
                for bi, bt in enumerate(batches):
                    bt()
                    while wi < nw and (wi + 1) * nb <= (bi + 1) * nw:
                        warm[wi]()
                        wi += 1
                while wi < nw:
                    warm[wi]()
                    wi += 1
            for t in make_o_thunks(NSC - 1):
                t()

    nc.compile()
    return nc


def kernel(
    q,
    k,
    v,
    mask=None,
    Wq=None,
    bq=None,
    Wk=None,
    bk=None,
    Wv=None,
    bv=None,
    Wo=None,
    bo=None,
    **_unused,
):
    from concourse.bass_utils import run_bass_kernel_spmd

    if "nc" not in _cache:
        _cache["nc"] = _build()
    nc = _cache["nc"]

    q = np.asarray(q, np.float32)
    k = np.asarray(k, np.float32)
    v = np.asarray(v, np.float32)
    Wq = np.asarray(Wq, np.float32)
    Wk = np.asarray(Wk, np.float32)
    Wv = np.asarray(Wv, np.float32)
    Wo = np.asarray(Wo, np.float32)
    bq = np.zeros(D, np.float32) if bq is None else np.asarray(bq, np.float32)
    bk = np.zeros(D, np.float32) if bk is None else np.asarray(bk, np.float32)
    bv = np.zeros(D, np.float32) if bv is None else np.asarray(bv, np.float32)
    bo = np.zeros(D, np.float32) if bo is None else np.asarray(bo, np.float32)

    qr, kr, vr = _round_f32r(q), _round_f32r(k), _round_f32r(v)
    Wqr, Wkr, Wvr, Wor = map(_round_f32r, (Wq, Wk, Wv, Wo))

    # master causal mask: m[kk, c] = 1 iff kk <= c - 384
    kk = np.arange(128)[:, None]
    cc = np.arange(896)[None, :]
    masks = (kk <= cc - 384).astype(np.float32)

    xT = {}
    for b in range(B):
        xT[("q", b)] = np.ascontiguousarray(qr[b].T)
        xT[("k", b)] = np.ascontiguousarray(kr[b].T)
        xT[("v", b)] = np.ascontiguousarray(vr[b].T)
    wqs, wks, wvs, wos, bqs, bks = {}, {}, {}, {}, {}, {}
    for hg in range(2):
        sl = slice(hg * FH, (hg + 1) * FH)
        wqs[hg] = np.ascontiguousarray(Wqr[sl, :].T) * np.float32(0.125)
        wks[hg] = np.ascontiguousarray(Wkr[sl, :].T)
        wvs[hg] = np.ascontiguousarray(Wvr[sl, :].T)
        wos[hg] = np.ascontiguousarray(Wor[:, sl].T)
        bqs[hg] = np.ascontiguousarray(bq[sl]) * np.float32(0.125)
        bks[hg] = np.ascontiguousarray(bk[sl])

    in_maps = []
    for c in range(NCORES):
        b, hg = c // 2, c % 2
        in_maps.append(
            {
                "xq": xT[("q", b)],
                "xk": xT[("k", b)],
                "xv": xT[("v", b)],
                "wq": wqs[hg],
                "wk": wks[hg],
                "wv": wvs[hg],
                "wo": wos[hg],
                "masks": masks,
                "bq": bqs[hg],
                "bk": bks[hg],
            }
        )

    res = run_bass_kernel_spmd(nc, in_maps, list(range(NCORES)))
    out = np.empty((B, S, D), np.float32)
    for b in range(B):
        out[b] = res.results[2 * b]["out"] + res.results[2 * b + 1]["out"]
    const = Wo @ bv + bo  # bv/bo contribution (folds exactly through softmax)
    if np.any(const):
        out += const[None, None, :]
    return out



# revision 4
# speedup vs baseline: 1.4520x; 1.4520x over previous
"""Multi-head causal attention (B=4, S=2048, D=1024, H=16) on 8 trn2 cores.

Sharding: tensor-parallel over heads x data-parallel over batch.
core c -> (batch b = c//2, head-group hg = c%2 of 8 heads). Every core runs
an identical SPMD program on different data. Host sums the two partial
outputs per batch and folds the Wo @ bv + bo constant.

v2 design (vs f32r baseline at ~470us):
  - All matmul data is bf16 (tolerance is 2e-2; bf16 lands ~2e-3). bf16
    enables Fast Weight Load (f32r is fp32-class -> no FWL), which removes
    the ~60ns/MM LDWEIGHTS overhead observed in the baseline trace.
  - Score matmuls are K=128 zero-padded: per-head K^T tiles (ktp) hold the
    64 dk rows in the half of the partition range matching the head's rows
    in the packed Q tile, zeros elsewhere. Zeros contribute exactly 0, and
    every matmul is a full-K=128 N=512 stream => PE activity monitor (HAM)
    sees a dense stream and holds the 2.4 GHz clock.
  - Weights resident in SBUF (loaded once); ctx kept in SBUF (no DRAM
    spill/reload of the per-head context).
  - Attention processed in 2-key-block slabs: scores into a [128,1024]
    2-bank PSUM tile, ONE exp per slab (FD=1024 amortizes ScalarE's
    ~352-cycle per-instruction overhead), masked diag slabs via one bf16
    DVE multiply against a precomputed [128,2048] slab mask.
  - Softmax denominators via the ones-column trick in the PV stationary
    (V augmented to 65 cols/head); normalization = DVE reciprocal of the
    PSUM denominator row -> gpsimd partition-broadcast -> DVE multiply,
    written straight into the SBUF ctx tiles.
  - Emission is software-pipelined: scores(i+1) and a projection/output
    filler chunk are emitted between scores(i) and PV(i), so the PE never
    waits on the exp; QKV projections for the next superblock and the
    output projection for the previous one ride along as filler.
"""

import sys

import numpy as np

_BASS_PATH = "/opt/trn_rl_repo"
if _BASS_PATH not in sys.path:
    sys.path.insert(0, _BASS_PATH)

B, S, D, H, DK = 4, 2048, 1024, 16, 64
NCORES = 8
FH = 512  # features per core (8 heads)
HL = 8  # local heads
NSC = 4  # seq superblocks of 512
SQ = 512
NKB = 16  # key blocks of 128
NDM = 8  # d_model chunks of 128

_cache = {}


def _build():
    import concourse.bacc as bacc
    import concourse.mybir as mybir
    from concourse.tile import TileContext

    f32, bf16 = mybir.dt.float32, mybir.dt.bfloat16
    AF = mybir.ActivationFunctionType

    nc = bacc.Bacc("TRN2", target_bir_lowering=False, debug=False, num_devices=1)

    xq_d = nc.dram_tensor("xq", [D, S], bf16, kind="ExternalInput").ap()
    xk_d = nc.dram_tensor("xk", [D, S], bf16, kind="ExternalInput").ap()
    xv_d = nc.dram_tensor("xv", [D, S], bf16, kind="ExternalInput").ap()
    wq_d = nc.dram_tensor("wq", [D, FH], bf16, kind="ExternalInput").ap()
    wk_d = nc.dram_tensor("wk", [D, FH], bf16, kind="ExternalInput").ap()
    wv_d = nc.dram_tensor("wv", [D, FH], bf16, kind="ExternalInput").ap()
    wo_d = nc.dram_tensor("wo", [FH, D], bf16, kind="ExternalInput").ap()
    # slab masks [128, 2048]: cols 1024*d + 512*p + q hold the 0/1 causal
    # mask for diagonal key-block j = 2d+p: m = (k <= q - 128*j)
    mask_d = nc.dram_tensor("masks", [128, 2048], bf16, kind="ExternalInput").ap()
    bq_d = nc.dram_tensor("bq", [FH], f32, kind="ExternalInput").ap()
    bk_d = nc.dram_tensor("bk", [FH], f32, kind="ExternalInput").ap()
    out_d = nc.dram_tensor("out", [S, D], f32, kind="ExternalOutput").ap()

    with TileContext(nc) as tc:
        with (
            tc.tile_pool(name="res", bufs=1) as res,
            tc.tile_pool(name="st", bufs=1) as st,
            tc.tile_pool(name="psum", bufs=1, space="PSUM") as psp,
        ):
            # ---- resident tiles ----
            # per-head K^T, zero-padded to K=128: even heads use partitions
            # 0-63 (matching their rows in the packed Q tile), odd heads
            # 64-127; the other half stays zero.
            ktp = [
                res.tile([128, S], bf16, name=f"ktp{h}", tag=f"ktp{h}")
                for h in range(HL)
            ]
            for h in range(HL):
                z = slice(64, 128) if h % 2 == 0 else slice(0, 64)
                nc.vector.memset(ktp[h][z, :], 0.0)
            # V augmented: 8 heads x (64 V cols + ones col) + pad
            vaug = [
                res.tile([128, 584], bf16, name=f"va{k}", tag=f"va{k}")
                for k in range(NKB)
            ]
            for k in range(NKB):
                # pad cols (past the 8*65 data cols) are read by head 7's
                # 128-wide PV stationary slice; zero them once
                nc.vector.memset(vaug[k][:, HL * 65 : 584], 0.0)
            mask_sb = res.tile([128, 2048], bf16, name="mask_sb", tag="mask_sb")
            nc.sync.dma_start(mask_sb[:], mask_d[:])
            bq_t = [res.tile([128, 1], f32, name=f"bq{i}", tag=f"bq{i}") for i in range(4)]
            bk_t = [res.tile([128, 1], f32, name=f"bk{i}", tag=f"bk{i}") for i in range(4)]
            for i in range(4):
                nc.sync.dma_start(
                    bq_t[i][:],
                    bq_d[i * 128 : (i + 1) * 128].rearrange("(p o) -> p o", o=1),
                )
                nc.sync.dma_start(
                    bk_t[i][:],
                    bk_d[i * 128 : (i + 1) * 128].rearrange("(p o) -> p o", o=1),
                )
            ones_t = res.tile([128, HL], f32, name="ones", tag="ones")
            nc.vector.memset(ones_t[:], 1.0)
            # resident weights
            wq_sb, wk_sb, wv_sb = [], [], []
            for pname, w_d, lst in (
                ("q", wq_d, wq_sb),
                ("k", wk_d, wk_sb),
                ("v", wv_d, wv_sb),
            ):
                for dm in range(NDM):
                    wt = res.tile(
                        [128, FH], bf16, name=f"w{pname}{dm}", tag=f"w{pname}{dm}"
                    )
                    nc.sync.dma_start(wt[:], w_d[dm * 128 : (dm + 1) * 128, :])
                    lst.append(wt)
            wo_sb = []
            for fc in range(4):
                wt = res.tile([128, D], bf16, name=f"wo{fc}", tag=f"wo{fc}")
                nc.sync.dma_start(wt[:], wo_d[fc * 128 : (fc + 1) * 128, :])
                wo_sb.append(wt)
            # per-superblock packed Q (2 heads per tile), resident
            qs = [
                [
                    res.tile([128, SQ], bf16, name=f"qs{sc}_{i}", tag=f"qs{sc}_{i}")
                    for i in range(4)
                ]
                for sc in range(NSC)
            ]
            # ctx^T in SBUF: 4 tiles [128 feats, S]
            ctx = [
                res.tile([128, S], bf16, name=f"ctx{fc}", tag=f"ctx{fc}")
                for fc in range(4)
            ]

            # ---- filler units (projection / output-projection chunks) ----
            def make_proj_units(sc):
                """QKV projections for superblock sc, as ~4-MM units."""
                units = []
                for pname, x_d, w_sb in (
                    ("k", xk_d, wk_sb),
                    ("v", xv_d, wv_sb),
                    ("q", xq_d, wq_sb),
                ):
                    box = {}

                    def load(pname=pname, x_d=x_d, box=box, sc=sc):
                        xr = []
                        for dm in range(NDM):
                            xt = st.tile(
                                [128, SQ],
                                bf16,
                                name=f"x{dm}",
                                tag=f"x{dm}",
                                bufs=2,
                            )
                            nc.sync.dma_start(
                                xt[:],
                                x_d[dm * 128 : (dm + 1) * 128, sc * SQ : (sc + 1) * SQ],
                            )
                            xr.append(xt)
                        box["x"] = xr

                    for gi in range(4):
                        for half in range(2):

                            def unit(
                                pname=pname,
                                w_sb=w_sb,
                                gi=gi,
                                half=half,
                                box=box,
                                sc=sc,
                                load=load,
                            ):
                                if gi == 0 and half == 0:
                                    load()
                                xr = box["x"]
                                if half == 0:
                                    if pname == "v":
                                        box["pp"] = psp.tile(
                                            [128, FH], f32, name="pp", tag="pp", bufs=2
                                        )
                                    else:
                                        box["pp"] = psp.tile(
                                            [128, SQ], f32, name="pp", tag="pp", bufs=2
                                        )
                                pp = box["pp"]
                                dms = range(4 * half, 4 * half + 4)
                                if pname in ("q", "k"):
                                    for dm in dms:
                                        nc.tensor.matmul(
                                            pp[:],
                                            w_sb[dm][:, gi * 128 : (gi + 1) * 128],
                                            xr[dm][:],
                                            start=(dm == 0),
                                            stop=(dm == NDM - 1),
                                        )
                                else:
                                    for dm in dms:
                                        nc.tensor.matmul(
                                            pp[:],
                                            xr[dm][:, gi * 128 : (gi + 1) * 128],
                                            w_sb[dm][:],
                                            start=(dm == 0),
                                            stop=(dm == NDM - 1),
                                        )
                                if half == 0:
                                    return
                                # evict
                                if pname == "k":
                                    # split per head into zero-padded ktp
                                    h0, h1 = 2 * gi, 2 * gi + 1
                                    nc.scalar.activation(
                                        ktp[h0][0:64, sc * SQ : (sc + 1) * SQ],
                                        pp[0:64, :],
                                        AF.Identity,
                                        bias=bk_t[gi][0:64],
                                    )
                                    nc.scalar.activation(
                                        ktp[h1][64:128, sc * SQ : (sc + 1) * SQ],
                                        pp[64:128, :],
                                        AF.Identity,
                                        bias=bk_t[gi][64:128],
                                    )
                                elif pname == "q":
                                    nc.scalar.activation(
                                        qs[sc][gi][:],
                                        pp[:],
                                        AF.Identity,
                                        bias=bq_t[gi][:],
                                    )
                                else:  # v
                                    kb = sc * 4 + gi
                                    va3 = vaug[kb][:, 0 : HL * 65].rearrange(
                                        "p (h e) -> p h e", e=65
                                    )
                                    pp3 = pp[:].rearrange("p (h e) -> p h e", e=64)
                                    nc.vector.tensor_copy(va3[:, :, 0:64], pp3[:])
                                    nc.vector.tensor_copy(
                                        va3[:, :, 64:65],
                                        ones_t[:].rearrange("p (h o) -> p h o", o=1),
                                    )

                            units.append(unit)
                return units

            def make_o_units(sb):
                """Output projection for superblock sb: 8 units of 4 MMs."""
                units = []
                for qb in range(4):
                    for n2 in range(2):

                        def unit(qb=qb, n2=n2, sb=sb):
                            pp = psp.tile([128, SQ], f32, name="pp", tag="pp", bufs=2)
                            for fc in range(4):
                                nc.tensor.matmul(
                                    pp[:],
                                    ctx[fc][:, sb * SQ + qb * 128 : sb * SQ + (qb + 1) * 128],
                                    wo_sb[fc][:, n2 * SQ : (n2 + 1) * SQ],
                                    start=(fc == 0),
                                    stop=(fc == 3),
                                )
                            ob = st.tile([128, SQ], f32, name="ob", tag="ob", bufs=2)
                            nc.vector.tensor_copy(ob[:], pp[:])
                            nc.sync.dma_start(
                                out_d[
                                    sb * SQ + qb * 128 : sb * SQ + (qb + 1) * 128,
                                    n2 * SQ : (n2 + 1) * SQ,
                                ],
                                ob[:],
                            )

                        units.append(unit)
                return units

            # ---- attention ----
            def attn_emit(h, sb, filler):
                """Emit attention for (head h, superblock sb), pipelined.

                filler: callable that emits ~4 matmuls of independent work
                when invoked (or nothing if exhausted).
                """
                ti = h // 2
                nkb = 4 * (sb + 1)
                # non-diagonal slabs first, diagonal (masked) last
                kbs = list(range(0, 4 * sb)) + list(range(4 * sb, 4 * sb + 4))
                slabs = [(kbs[i], kbs[i + 1]) for i in range(0, nkb, 2)]
                nsl = len(slabs)
                cp = psp.tile([128, SQ], f32, name="cp", tag="cp", bufs=2)

                es_tiles = [None] * nsl

                def emit_scores(i):
                    kb0, kb1 = slabs[i]
                    sp = psp.tile([128, 2 * SQ], f32, name="sp", tag="sp", bufs=2)
                    for p, kb in enumerate((kb0, kb1)):
                        nc.tensor.matmul(
                            sp[:, p * SQ : (p + 1) * SQ],
                            ktp[h][:, kb * 128 : (kb + 1) * 128],
                            qs[sb][ti][:],
                            start=True,
                            stop=True,
                        )
                    es = st.tile([128, 2 * SQ], bf16, name="es", tag="es", bufs=4)
                    nc.scalar.activation(es[:], sp[:], AF.Exp)
                    if kb0 >= 4 * sb:
                        d = (kb0 - 4 * sb) // 2
                        es2 = st.tile(
                            [128, 2 * SQ], bf16, name="es2", tag="es2", bufs=2
                        )
                        nc.vector.tensor_mul(
                            es2[:], es[:], mask_sb[:, d * 1024 : (d + 1) * 1024]
                        )
                        es = es2
                    es_tiles[i] = es

                def emit_pv(i):
                    kb0, kb1 = slabs[i]
                    es = es_tiles[i]
                    for p, kb in enumerate((kb0, kb1)):
                        nc.tensor.matmul(
                            cp[:],
                            vaug[kb][:, h * 65 : h * 65 + 128],
                            es[:, p * SQ : (p + 1) * SQ],
                            start=(i == 0 and p == 0),
                            stop=(i == nsl - 1 and p == 1),
                        )

                emit_scores(0)
                for i in range(nsl):
                    if i + 1 < nsl:
                        emit_scores(i + 1)
                    filler()
                    emit_pv(i)
                # normalization: 1/denominator broadcast over the 64 V rows.
                # Stage the PSUM denominator row to SBUF first: custom-DVE
                # ops (reciprocal_approx_fast) need SBUF operands on HW.
                d1 = st.tile([1, SQ], f32, name="d1", tag="d1", bufs=2)
                nc.vector.tensor_copy(d1[:], cp[64:65, :])
                rc1 = st.tile([1, SQ], f32, name="rc1", tag="rc1", bufs=2)
                nc.vector.reciprocal_approx_fast(rc1[:], d1[:])
                rb = st.tile([64, SQ], f32, name="rb", tag="rb", bufs=2)
                nc.gpsimd.partition_broadcast(rb[:], rc1[:])
                nc.vector.tensor_mul(
                    ctx[ti][(h % 2) * 64 : (h % 2) * 64 + 64, sb * SQ : (sb + 1) * SQ],
                    cp[0:64, :],
                    rb[:],
                )

            # ---- emission schedule ----
            for u in make_proj_units(0):
                u()
            for sb in range(NSC):
                units = []
                if sb < NSC - 1:
                    units += make_proj_units(sb + 1)
                if sb >= 1:
                    units += make_o_units(sb - 1)
                nslabs = HL * 2 * (sb + 1)
                state = {"ui": 0, "si": 0}

                def filler(units=units, state=state, nslabs=nslabs):
                    state["si"] += 1
                    nu = len(units)
                    while (
                        state["ui"] < nu
                        and (state["ui"] + 1) * nslabs <= state["si"] * nu
                    ):
                        units[state["ui"]]()
                        state["ui"] += 1

                for h in range(HL):
                    attn_emit(h, sb, filler)
                while state["ui"] < len(units):
                    units[state["ui"]]()
                    state["ui"] += 1
            for u in make_o_units(NSC - 1):
                u()

    nc.compile()
    return nc


def kernel(
    q,
    k,
    v,
    mask=None,
    Wq=None,
    bq=None,
    Wk=None,
    bk=None,
    Wv=None,
    bv=None,
    Wo=None,
    bo=None,
    **_unused,
):
    import ml_dtypes
    from concourse.bass_utils import run_bass_kernel_spmd

    if "nc" not in _cache:
        _cache["nc"] = _build()
    nc = _cache["nc"]

    bf16 = ml_dtypes.bfloat16
    q = np.asarray(q, np.float32)
    k = np.asarray(k, np.float32)
    v = np.asarray(v, np.float32)
    Wq = np.asarray(Wq, np.float32)
    Wk = np.asarray(Wk, np.float32)
    Wv = np.asarray(Wv, np.float32)
    Wo = np.asarray(Wo, np.float32)
    bq = np.zeros(D, np.float32) if bq is None else np.asarray(bq, np.float32)
    bk = np.zeros(D, np.float32) if bk is None else np.asarray(bk, np.float32)
    bv = np.zeros(D, np.float32) if bv is None else np.asarray(bv, np.float32)
    bo = np.zeros(D, np.float32) if bo is None else np.asarray(bo, np.float32)

    # slab masks: m2[k, 1024*d + 512*p + q] = 1 iff k <= q - 128*(2d+p)
    kk = np.arange(128)[:, None]
    masks = np.empty((128, 2048), np.float32)
    qq = np.arange(512)[None, :]
    for d in range(2):
        for p in range(2):
            j = 2 * d + p
            masks[:, 1024 * d + 512 * p : 1024 * d + 512 * p + 512] = (
                kk <= qq - 128 * j
            ).astype(np.float32)
    masks = masks.astype(bf16)

    xT = {}
    for b in range(B):
        xT[("q", b)] = np.ascontiguousarray(q[b].T).astype(bf16)
        xT[("k", b)] = np.ascontiguousarray(k[b].T).astype(bf16)
        xT[("v", b)] = np.ascontiguousarray(v[b].T).astype(bf16)
    wqs, wks, wvs, wos, bqs, bks = {}, {}, {}, {}, {}, {}
    for hg in range(2):
        sl = slice(hg * FH, (hg + 1) * FH)
        wqs[hg] = (np.ascontiguousarray(Wq[sl, :].T) * np.float32(0.125)).astype(bf16)
        wks[hg] = np.ascontiguousarray(Wk[sl, :].T).astype(bf16)
        wvs[hg] = np.ascontiguousarray(Wv[sl, :].T).astype(bf16)
        wos[hg] = np.ascontiguousarray(Wo[:, sl].T).astype(bf16)
        bqs[hg] = np.ascontiguousarray(bq[sl]) * np.float32(0.125)
        bks[hg] = np.ascontiguousarray(bk[sl])

    in_maps = []
    for c in range(NCORES):
        b, hg = c // 2, c % 2
        in_maps.append(
            {
                "xq": xT[("q", b)],
                "xk": xT[("k", b)],
                "xv": xT[("v", b)],
                "wq": wqs[hg],
                "wk": wks[hg],
                "wv": wvs[hg],
                "wo": wos[hg],
                "masks": masks,
                "bq": bqs[hg],
                "bk": bks[hg],
            }
        )

    res = run_bass_kernel_spmd(nc, in_maps, list(range(NCORES)))
    out = np.empty((B, S, D), np.float32)
    for b in range(B):
        out[b] = res.results[2 * b]["out"] + res.results[2 * b + 1]["out"]
    const = Wo @ bv + bo  # bv/bo contribution (folds exactly through softmax)
    if np.any(const):
        out += const[None, None, :]
    return out
